# revision 16
# baseline (speedup 1.0000x reference)
"""DeepClusterLoss on 8 Trainium2 NeuronCores (Bass/Tile).

reference:
    recon_loss   = sum((recon_x - x)**2)
    cluster_loss = sum((x - centers[assign])**2)
    total        = recon_loss + cluster_loss          (ALPHA = BETA = 1)

Decomposition:
    cluster_loss = sum|x|^2 - 2*sum_k <S_k, C_k> + sum_k n_k*|C_k|^2
where S_k is the per-cluster segment sum of x and n_k the counts (host-side
bincount, which the cluster-sort requires anyway).

Device strategy (data-parallel over N):
  - Host sorts each core's samples by cluster id and pads every cluster to
    J*128 rows (J=11; capacity 1408 >= actual max count 1358).  Every PAIR
    of 128-sample slots then belongs to one cluster, so a single
    self-loading matmul per pair against a constant ones-vector
        ps[:, g] += x_pair[128, 2*64].T @ ones[128, 1]
    accumulates S_g in PSUM (rows 0:64 = even slot dims, rows 64:128 = odd
    slot dims; host adds the halves).  The 128-column bf16 stationary takes
    the fast-weight-load path (~63 ns per LDWEIGHTS+MATMUL pair measured).
  - J=11 is odd, so each cluster has 5 in-cluster pairs plus a BOUNDARY
    pair (its slot 10 + the next cluster's slot 0) that accumulates into a
    second PSUM bank whose top half the host discards.  Tiles carry one
    overlap slot so boundary pairs never cross a DMA tile.
  - Streams are bf16.  DVE computes d = r - x and x^2 (2x bf16 mode); ACT
    squares d with fused per-partition accumulation; x^2 is reduced by the
    same pair-matmul trick into two more PSUM banks.
  - Host combines the tiny per-core outputs in float64.
"""

import sys
from contextlib import ExitStack

import numpy as np

for _p in ("/opt/trn_rl_repo", "/opt/pypackages"):
    if _p not in sys.path:
        sys.path.append(_p)

import ml_dtypes
import concourse.tile as tile
from concourse import bacc, mybir
from concourse.bass_utils import run_bass_kernel_spmd

N, D, K = 1_000_000, 64, 100
ALPHA, BETA = 1.0, 1.0
N_CORES = 8
N_PER_CORE = N // N_CORES   # 125000
P = 128                     # SBUF partitions = samples per slot
J = 11                      # slots per cluster (capacity 1408 >= max 1358)
SLOTS_TOTAL = K * J         # 1100 slots per core
PSLOTS = SLOTS_TOTAL + 1    # +1 zero pad slot: cluster 99's boundary partner
PADDED = SLOTS_TOTAL * P    # 140800 rows per core
# Per-tile slot counts (multiples of J so cluster boundaries align; tapered
# first/last tiles shrink the pipeline ramp and tail).
TILE_SLOTS = [33, 22] + [55] * 18 + [33, 22]
assert sum(TILE_SLOTS) == SLOTS_TOTAL
NTILES = len(TILE_SLOTS)    # 22
TILE_OFF = np.concatenate([[0], np.cumsum(TILE_SLOTS)])[:-1]

_bf16 = mybir.dt.bfloat16
_f32 = mybir.dt.float32
BF16 = ml_dtypes.bfloat16


def build_nc():
    """Build + compile the per-core Bass program (same program on all cores)."""
    nc = bacc.Bacc()
    # Per-tile input: [P, (ns+1)*D x-cols (incl. overlap slot) | ns*D r-cols]
    xr_d = [
        nc.dram_tensor(f"xr{t}", [P, (2 * ns + 1) * D], _bf16, kind="ExternalInput")
        for t, ns in enumerate(TILE_SLOTS)
    ]
    # s_out[0:64,k] + s_out[64:128,k] + sb_out[0:64,k] = S_k
    s_out = nc.dram_tensor("s_out", [P, K], _f32, kind="ExternalOutput")
    sb_out = nc.dram_tensor("sb_out", [P, K], _f32, kind="ExternalOutput")
    # sum(s2_out) + sum(s2b_out[0:64,:]) = sum of x^2
    s2_out = nc.dram_tensor("s2_out", [P, K], _f32, kind="ExternalOutput")
    s2b_out = nc.dram_tensor("s2b_out", [P, K], _f32, kind="ExternalOutput")
    part_out = nc.dram_tensor("partials", [P, NTILES], _f32, kind="ExternalOutput")

    with ExitStack() as ctx:
        tc = ctx.enter_context(tile.TileContext(nc))
        const_pool = ctx.enter_context(tc.tile_pool(name="const", bufs=1))
        xin = ctx.enter_context(tc.tile_pool(name="xin", bufs=3))
        dp = ctx.enter_context(tc.tile_pool(name="dp", bufs=2))
        sqp = ctx.enter_context(tc.tile_pool(name="sqp", bufs=2))
        x2p = ctx.enter_context(tc.tile_pool(name="x2p", bufs=2))
        psum = ctx.enter_context(tc.tile_pool(name="psum", bufs=1, space="PSUM"))

        ones1 = const_pool.tile([P, 1], _bf16)
        nc.vector.memset(ones1[:], 1.0)
        partials_sb = const_pool.tile([P, NTILES], _f32)

        ps = psum.tile([P, K], _f32, tag="ps")     # in-cluster pairs of x
        psb = psum.tile([P, K], _f32, tag="psb")   # boundary pairs of x
        ps2 = psum.tile([P, K], _f32, tag="ps2")   # in-cluster pairs of x^2
        ps2b = psum.tile([P, K], _f32, tag="ps2b")  # boundary pairs of x^2

        for t, ns in enumerate(TILE_SLOTS):
            nx = (ns + 1) * D               # x cols incl. overlap slot
            xr_t = xin.tile([P, (2 * ns + 1) * D], _bf16, tag="xr")
            # split the load across both HWDGE rings
            nc.sync.dma_start(xr_t[:, 0:nx], xr_d[t][:, 0:nx])
            nc.scalar.dma_start(xr_t[:, nx:], xr_d[t][:, nx:])
            x_t = xr_t[:, 0:nx]             # [P, (ns+1)*D]
            r_t = xr_t[:, nx:]              # [P, ns*D]

            # x^2 elementwise (incl. overlap slot); PE reduces it below
            x2_t = x2p.tile([P, nx], _bf16, tag="x2")
            nc.vector.tensor_mul(x2_t[:], x_t, x_t)

            d_t = dp.tile([P, ns * D], _bf16, tag="d")
            nc.vector.tensor_sub(d_t[:], r_t, xr_t[:, 0 : ns * D])
            sq_t = sqp.tile([P, ns * D], _bf16, tag="sq")
            nc.scalar.activation(
                sq_t[:], d_t[:], mybir.ActivationFunctionType.Square,
                accum_out=partials_sb[:, t : t + 1],
            )

            for c in range(ns // J):        # clusters in this tile
                g = int(TILE_OFF[t]) // J + c
                base = c * J
                for q in range(5):          # in-cluster pairs
                    lo = (base + 2 * q) * D
                    nc.tensor.matmul(
                        ps[:, g : g + 1], xr_t[:, lo : lo + 2 * D], ones1[:],
                        start=(q == 0), stop=(q == 4),
                    )
                    nc.tensor.matmul(
                        ps2[:, g : g + 1], x2_t[:, lo : lo + 2 * D], ones1[:],
                        start=(q == 0), stop=(q == 4),
                    )
                lo = (base + 10) * D        # boundary pair (top half junk)
                nc.tensor.matmul(
                    psb[:, g : g + 1], xr_t[:, lo : lo + 2 * D], ones1[:],
                    start=True, stop=True,
                )
                nc.tensor.matmul(
                    ps2b[:, g : g + 1], x2_t[:, lo : lo + 2 * D], ones1[:],
                    start=True, stop=True,
                )

        for name, bank, dram in (
            ("s_sb", ps, s_out), ("sb_sb", psb, sb_out),
            ("s2_sb", ps2, s2_out), ("s2b_sb", ps2b, s2b_out),
        ):
            sb = const_pool.tile([P, K], _f32, tag=name)
            nc.vector.tensor_copy(sb[:], bank[:])
            nc.sync.dma_start(dram[:, :], sb[:])
        nc.sync.dma_start(part_out[:, :], partials_sb[:])

    nc.compile()
    return nc


def host_prepare(recon_x, x, cluster_assignments):
    """Shard, cluster-sort, pad, cast, and lay out the inputs per core."""
    x_np = np.asarray(x, dtype=np.float32).reshape(N_CORES, N_PER_CORE, D)
    r_np = np.asarray(recon_x, dtype=np.float32).reshape(N_CORES, N_PER_CORE, D)
    a_np = np.asarray(cluster_assignments).reshape(N_CORES, N_PER_CORE)
    a_np = a_np.astype(np.int64)

    in_maps = []
    counts = np.zeros((N_CORES, K), np.int64)
    for c in range(N_CORES):
        a = a_np[c]
        cnt = np.bincount(a, minlength=K)
        counts[c] = cnt
        assert cnt.max() <= J * P, f"cluster overflow: {cnt.max()} > {J * P}"
        starts = np.zeros(K, np.int64)
        starts[1:] = np.cumsum(cnt)[:-1]
        order = np.argsort(a, kind="stable")
        g_sorted = a[order]
        dest = g_sorted * (J * P) + (np.arange(N_PER_CORE) - starts[g_sorted])

        # slot-major views [PSLOTS, P, D]; slot SLOTS_TOTAL stays all-zero
        xp = np.zeros((PSLOTS, P, D), BF16)
        rp = np.zeros((PSLOTS, P, D), BF16)
        xp.reshape(-1, D)[dest] = x_np[c][order].astype(BF16)
        rp.reshape(-1, D)[dest] = r_np[c][order].astype(BF16)

        im = {}
        for t, ns in enumerate(TILE_SLOTS):
            o = int(TILE_OFF[t])
            buf = np.empty((P, (2 * ns + 1) * D), BF16)
            buf[:, 0 : (ns + 1) * D] = (
                xp[o : o + ns + 1].transpose(1, 0, 2).reshape(P, (ns + 1) * D)
            )
            buf[:, (ns + 1) * D :] = (
                rp[o : o + ns].transpose(1, 0, 2).reshape(P, ns * D)
            )
            im[f"xr{t}"] = buf
        in_maps.append(im)
    return in_maps, counts


def host_combine(results, counts, cluster_centers):
    """Reduce per-core outputs into (total, recon, cluster) in float64."""
    S = np.zeros((K, D), np.float64)
    x2 = 0.0
    recon = 0.0
    for rd in results:
        so = rd["s_out"].astype(np.float64)
        sb = rd["sb_out"].astype(np.float64)
        S += (so[0:D, :] + so[D : 2 * D, :] + sb[0:D, :]).T
        x2 += rd["s2_out"].astype(np.float64).sum()
        x2 += rd["s2b_out"].astype(np.float64)[0:D, :].sum()
        recon += rd["partials"].astype(np.float64).sum()
    C = np.asarray(cluster_centers, dtype=np.float64)
    cross = float((S * C).sum())
    n_k = counts.sum(axis=0).astype(np.float64)
    w = float((n_k * (C * C).sum(axis=1)).sum())
    cluster = x2 - 2.0 * cross + w
    total = ALPHA * recon + BETA * cluster
    return (np.float32(total), np.float32(recon), np.float32(cluster))


_nc = None


def _get_nc():
    global _nc
    if _nc is None:
        _nc = build_nc()
    return _nc


def kernel(recon_x, x, cluster_assignments, cluster_centers):
    nc = _get_nc()
    in_maps, counts = host_prepare(recon_x, x, cluster_assignments)
    res = run_bass_kernel_spmd(nc, in_maps, list(range(N_CORES)))
    return host_combine(res.results, counts, cluster_centers)


# revision 17
# speedup vs baseline: 1.2673x; 1.2673x over previous
"""DeepClusterLoss on 8 Trainium2 NeuronCores (Bass/Tile).

reference:
    recon_loss   = sum((recon_x - x)**2)
    cluster_loss = sum((x - centers[assign])**2)
    total        = recon_loss + cluster_loss          (ALPHA = BETA = 1)

Decomposition:
    cluster_loss = sum|x|^2 - 2*sum_k <S_k, C_k> + sum_k n_k*|C_k|^2
where S_k is the per-cluster segment sum of x and n_k the counts (host-side
bincount, which the cluster-sort requires anyway).

Device strategy (data-parallel over N):
  - Host sorts each core's samples by cluster id and pads every cluster to
    J*128 rows (J=11; capacity 1408 >= actual max count 1358).  Every PAIR
    of 128-sample slots then belongs to one cluster, so a single
    self-loading matmul per pair against a constant ones-vector
        ps[:, g] += x_pair[128, 2*64].T @ ones[128, 1]
    accumulates S_g in PSUM (rows 0:64 = even slot dims, rows 64:128 = odd
    slot dims; host adds the halves).  The 128-column bf16 stationary takes
    the fast-weight-load path (~63 ns per LDWEIGHTS+MATMUL pair measured).
  - J=11 is odd, so each cluster has 5 in-cluster pairs plus a BOUNDARY
    pair (its slot 10 + the next cluster's slot 0) that accumulates into a
    second PSUM bank whose top half the host discards.  Tiles carry one
    overlap slot so boundary pairs never cross a DMA tile.
  - Streams are bf16.  DVE computes d = r - x and x^2 (2x bf16 mode); ACT
    squares d with fused per-partition accumulation; x^2 is reduced by the
    same pair-matmul trick into two more PSUM banks.
  - Host combines the tiny per-core outputs in float64.
"""

import sys
from contextlib import ExitStack

import numpy as np

for _p in ("/opt/trn_rl_repo", "/opt/pypackages"):
    if _p not in sys.path:
        sys.path.append(_p)

import ml_dtypes
import concourse.tile as tile
from concourse import bacc, mybir
from concourse.bass_utils import run_bass_kernel_spmd

N, D, K = 1_000_000, 64, 100
ALPHA, BETA = 1.0, 1.0
N_CORES = 8
N_PER_CORE = N // N_CORES   # 125000
P = 128                     # SBUF partitions = samples per slot
J = 11                      # slots per cluster (capacity 1408 >= max 1358)
SLOTS_TOTAL = K * J         # 1100 slots per core
PSLOTS = SLOTS_TOTAL + 1    # +1 zero pad slot: cluster 99's boundary partner
PADDED = SLOTS_TOTAL * P    # 140800 rows per core
# Per-tile slot counts (multiples of J so cluster boundaries align; tapered
# first/last tiles shrink the pipeline ramp and tail).
TILE_SLOTS = [33, 22] + [55] * 18 + [33, 22]
assert sum(TILE_SLOTS) == SLOTS_TOTAL
NTILES = len(TILE_SLOTS)    # 22
TILE_OFF = np.concatenate([[0], np.cumsum(TILE_SLOTS)])[:-1]

_bf16 = mybir.dt.bfloat16
_f32 = mybir.dt.float32
BF16 = ml_dtypes.bfloat16


def build_nc():
    """Build + compile the per-core Bass program (same program on all cores)."""
    nc = bacc.Bacc()
    # Per-tile input: [P, (ns+1)*D x-cols (incl. overlap slot) | ns*D r-cols]
    xr_d = [
        nc.dram_tensor(f"xr{t}", [P, (2 * ns + 1) * D], _bf16, kind="ExternalInput")
        for t, ns in enumerate(TILE_SLOTS)
    ]
    # s_out[0:64,k] + s_out[64:128,k] + sb_out[0:64,k] = S_k
    s_out = nc.dram_tensor("s_out", [P, K], _f32, kind="ExternalOutput")
    sb_out = nc.dram_tensor("sb_out", [P, K], _f32, kind="ExternalOutput")
    # sum(s2_out) + sum(s2b_out[0:64,:]) = sum of x^2
    s2_out = nc.dram_tensor("s2_out", [P, K], _f32, kind="ExternalOutput")
    s2b_out = nc.dram_tensor("s2b_out", [P, K], _f32, kind="ExternalOutput")
    part_out = nc.dram_tensor("partials", [P, NTILES], _f32, kind="ExternalOutput")

    with ExitStack() as ctx:
        tc = ctx.enter_context(tile.TileContext(nc))
        const_pool = ctx.enter_context(tc.tile_pool(name="const", bufs=1))
        xin = ctx.enter_context(tc.tile_pool(name="xin", bufs=3))
        dp = ctx.enter_context(tc.tile_pool(name="dp", bufs=2))
        sqp = ctx.enter_context(tc.tile_pool(name="sqp", bufs=2))
        x2p = ctx.enter_context(tc.tile_pool(name="x2p", bufs=2))
        psum = ctx.enter_context(tc.tile_pool(name="psum", bufs=1, space="PSUM"))

        ones1 = const_pool.tile([P, 1], _bf16)
        nc.vector.memset(ones1[:], 1.0)
        partials_sb = const_pool.tile([P, NTILES], _f32)

        ps = psum.tile([P, K], _f32, tag="ps")     # in-cluster pairs of x
        psb = psum.tile([P, K], _f32, tag="psb")   # boundary pairs of x
        ps2 = psum.tile([P, K], _f32, tag="ps2")   # in-cluster pairs of x^2
        ps2b = psum.tile([P, K], _f32, tag="ps2b")  # boundary pairs of x^2

        for t, ns in enumerate(TILE_SLOTS):
            nx = (ns + 1) * D               # x cols incl. overlap slot
            xr_t = xin.tile([P, (2 * ns + 1) * D], _bf16, tag="xr")
            nc.sync.dma_start(xr_t[:], xr_d[t][:, :])
            x_t = xr_t[:, 0:nx]             # [P, (ns+1)*D]
            r_t = xr_t[:, nx:]              # [P, ns*D]

            # x^2 elementwise (incl. overlap slot); PE reduces it below
            x2_t = x2p.tile([P, nx], _bf16, tag="x2")
            nc.vector.tensor_mul(x2_t[:], x_t, x_t)

            d_t = dp.tile([P, ns * D], _bf16, tag="d")
            nc.vector.tensor_sub(d_t[:], r_t, xr_t[:, 0 : ns * D])
            sq_t = sqp.tile([P, ns * D], _bf16, tag="sq")
            nc.scalar.activation(
                sq_t[:], d_t[:], mybir.ActivationFunctionType.Square,
                accum_out=partials_sb[:, t : t + 1],
            )

            for c in range(ns // J):        # clusters in this tile
                g = int(TILE_OFF[t]) // J + c
                base = c * J
                for q in range(5):          # in-cluster pairs
                    lo = (base + 2 * q) * D
                    nc.tensor.matmul(
                        ps[:, g : g + 1], xr_t[:, lo : lo + 2 * D], ones1[:],
                        start=(q == 0), stop=(q == 4),
                    )
                    nc.tensor.matmul(
                        ps2[:, g : g + 1], x2_t[:, lo : lo + 2 * D], ones1[:],
                        start=(q == 0), stop=(q == 4),
                    )
                lo = (base + 10) * D        # boundary pair (top half junk)
                nc.tensor.matmul(
                    psb[:, g : g + 1], xr_t[:, lo : lo + 2 * D], ones1[:],
                    start=True, stop=True,
                )
                nc.tensor.matmul(
                    ps2b[:, g : g + 1], x2_t[:, lo : lo + 2 * D], ones1[:],
                    start=True, stop=True,
                )

        for name, bank, dram in (
            ("s_sb", ps, s_out), ("sb_sb", psb, sb_out),
            ("s2_sb", ps2, s2_out), ("s2b_sb", ps2b, s2b_out),
        ):
            sb = const_pool.tile([P, K], _f32, tag=name)
            nc.vector.tensor_copy(sb[:], bank[:])
            nc.sync.dma_start(dram[:, :], sb[:])
        nc.sync.dma_start(part_out[:, :], partials_sb[:])

    nc.compile()
    return nc


def host_prepare(recon_x, x, cluster_assignments):
    """Shard, cluster-sort, pad, cast, and lay out the inputs per core."""
    x_np = np.asarray(x, dtype=np.float32).reshape(N_CORES, N_PER_CORE, D)
    r_np = np.asarray(recon_x, dtype=np.float32).reshape(N_CORES, N_PER_CORE, D)
    a_np = np.asarray(cluster_assignments).reshape(N_CORES, N_PER_CORE)
    a_np = a_np.astype(np.int64)

    in_maps = []
    counts = np.zeros((N_CORES, K), np.int64)
    for c in range(N_CORES):
        a = a_np[c]
        cnt = np.bincount(a, minlength=K)
        counts[c] = cnt
        assert cnt.max() <= J * P, f"cluster overflow: {cnt.max()} > {J * P}"
        starts = np.zeros(K, np.int64)
        starts[1:] = np.cumsum(cnt)[:-1]
        order = np.argsort(a, kind="stable")
        g_sorted = a[order]
        dest = g_sorted * (J * P) + (np.arange(N_PER_CORE) - starts[g_sorted])

        # slot-major views [PSLOTS, P, D]; slot SLOTS_TOTAL stays all-zero
        xp = np.zeros((PSLOTS, P, D), BF16)
        rp = np.zeros((PSLOTS, P, D), BF16)
        xp.reshape(-1, D)[dest] = x_np[c][order].astype(BF16)
        rp.reshape(-1, D)[dest] = r_np[c][order].astype(BF16)

        im = {}
        for t, ns in enumerate(TILE_SLOTS):
            o = int(TILE_OFF[t])
            buf = np.empty((P, (2 * ns + 1) * D), BF16)
            buf[:, 0 : (ns + 1) * D] = (
                xp[o : o + ns + 1].transpose(1, 0, 2).reshape(P, (ns + 1) * D)
            )
            buf[:, (ns + 1) * D :] = (
                rp[o : o + ns].transpose(1, 0, 2).reshape(P, ns * D)
            )
            im[f"xr{t}"] = buf
        in_maps.append(im)
    return in_maps, counts


def host_combine(results, counts, cluster_centers):
    """Reduce per-core outputs into (total, recon, cluster) in float64."""
    S = np.zeros((K, D), np.float64)
    x2 = 0.0
    recon = 0.0
    for rd in results:
        so = rd["s_out"].astype(np.float64)
        sb = rd["sb_out"].astype(np.float64)
        S += (so[0:D, :] + so[D : 2 * D, :] + sb[0:D, :]).T
        x2 += rd["s2_out"].astype(np.float64).sum()
        x2 += rd["s2b_out"].astype(np.float64)[0:D, :].sum()
        recon += rd["partials"].astype(np.float64).sum()
    C = np.asarray(cluster_centers, dtype=np.float64)
    cross = float((S * C).sum())
    n_k = counts.sum(axis=0).astype(np.float64)
    w = float((n_k * (C * C).sum(axis=1)).sum())
    cluster = x2 - 2.0 * cross + w
    total = ALPHA * recon + BETA * cluster
    return (np.float32(total), np.float32(recon), np.float32(cluster))


_nc = None


def _get_nc():
    global _nc
    if _nc is None:
        _nc = build_nc()
    return _nc


def kernel(recon_x, x, cluster_assignments, cluster_centers):
    nc = _get_nc()
    in_maps, counts = host_prepare(recon_x, x, cluster_assignments)
    res = run_bass_kernel_spmd(nc, in_maps, list(range(N_CORES)))
    return host_combine(res.results, counts, cluster_centers)
